# revision 11
# baseline (speedup 1.0000x reference)
"""CollaborativeAttention Trainium2 kernel.

Full inputs in, full output out. Shards batch (B=8) across 8 NeuronCores,
one batch element per core (no collectives). Matmuls are bf16 with fp32
PSUM accumulation, except the score path and the q/k input projections,
which run fp8 e4m3 with DoubleRow (2 MACs/cell/cycle); host-side upscales
(MIX_UPSCALE, QK_UPSCALE) keep fp8 operands out of the denormal range and
are divided back out inside the fused exp() scale.

The AV path also runs fp8 DoubleRow via a mean/residual split that works
because this regime's scores are tiny (softmax is near-uniform, exp~1):
  exp(z) @ v  =  colsum(v) + (exp(z)-1) @ v
The rank-1 colsum term (98% of the output norm) is computed once per core
from colsum(x) @ Wv in bf16/f32, and only the small residual
r = exp(z)-1 is quantized to fp8: its 2-3% quantization error scales
with the ~3%-of-norm signal term, not the total. r*64 is produced as
E - 64 where the ScalarE activation computes E = exp(z + ln 64) via its
bias input (there is no Expm1 on ScalarE); the subtract+fp8 cast runs on
the otherwise-idle GpSimd engine. v is quantized to e4m3 with a x32
upscale folded into Wv/bv on the host; 1/(64*32) plus the per-partition
colsum constant are applied in one fused DVE tensor_scalar when draining
the AV PSUM.

Per-core dataflow (batch element b), everything transposed so the feature
dim lives on partitions and no on-device transposes are ever needed:
  stage B (from host-pretransposed xT [C,N] and weights):
    qT[j,n]  = sum_c WqT[c,j] xT8[c,n]          (fp8 DoubleRow)
    kT[j,n]  = sum_c WkT[c,j] xT8[c,n]          (fp8 DoubleRow, kept bf16)
    v8[m,j'] = (sum_c xT[c,m] WvT_aug[c,j'] + bvB_aug) -> e4m3
               (j' = 12 blocks of [64 v-cols*32 | one 32-col]; the 32
               column makes the fp8 AV matmul emit the softmax-denominator
               residual; the last 12 columns of WvT_aug are Wcb*SCALE, so
               the content bias falls out of the same matmuls; ln64 is
               added when extracting it)
    c[j]     = colsum_v (from DVE-reduced colsum(x) @ WvT_aug + 1024*bv),
               round-tripped through DRAM to land as [65, H] on partitions
  per head h (emission software-pipelined: scores(h) then AV(h-1)):
    khT = kT * mix[h,:]   (DVE per-partition scalar, fp8 out)
    scoresT[m,n] psum = sum_d khT[d,m]^T qT8[d,n]     (fp8 DoubleRow)
    E[m,n] = exp(scale*scoresT + cb[m,h] + ln64)      (ScalarE, fused)
    r8[m,n] = E - 64 -> e4m3                          (GpSimd)
    po[65,n] psum = sum_m v8[m, block_h]^T r8[m,n]    (fp8 DoubleRow)
    pf = po/(64*32) + c[block_h]  (fused DVE tensor_scalar; row 64 = S[n])
    normalize off the PE path: S row -> DRAM -> broadcast-DMA to 64
    partitions -> reciprocal_approx_fast -> DVE multiply into ao. For the
    last two heads (kernel tail) the broadcast instead rides a 1-row f32r
    PE matmul (ones^T @ S) to cut the DRAM round-trip latency.
    Odd heads DMA-shift to partitions 64-127 so ao packs head PAIRS
    on 128 partitions (K=128 output projection with FWL).
  output projection: av(10) runs immediately after the head loop, then
  pairs 0..4 for all n-tiles (covering av(10)'s normalize latency), then
  the pair-5 tail accumulates via SBUF (+bproj), keeping the PE dense so
  it stays at full pstate through the tail.
  y is staged and stored bf16 (host casts back to f32) and its 8 output
  DMAs rotate across the sync/scalar/gpsimd issue queues.
"""

import numpy as np
import ml_dtypes

B, N, C = 8, 1024, 768
H, Dh = 12, 64
SCALE = Dh ** -0.5
NCORES = 8
BF16 = ml_dtypes.bfloat16

# fp8 (e4m3 + DoubleRow) for the score matmuls; k*mix is pre-scaled by
# MIX_UPSCALE on the host so values clear the e4m3 denormal floor, and the
# exp() scale divides it back out.
MIX_UPSCALE = 32.0
# fp8 DoubleRow for the q/k input projections; Wq/Wk are upscaled by
# QK_UPSCALE on the host (their ~0.02-scale values are denormal in e4m3),
# and the exp() scale divides the product back out.
QK_UPSCALE = 32.0
# fp8 DoubleRow AV path: v is upscaled x32 into e4m3 (folded into Wv/bv on
# the host); the residual r=exp-1 is upscaled x64 by computing
# E=exp(z+ln64) and subtracting 64. The AV drain divides by 64*32.
V_UPSCALE = 32.0
R_UPSCALE = 64.0
LN_R = float(np.log(R_UPSCALE))
AV_DIV = 1.0 / (V_UPSCALE * R_UPSCALE)

_CACHE = {}


def _chunks(total, size):
    out = []
    off = 0
    while off < total:
        out.append((off, min(size, total - off)))
        off += size
    return out


def emit(ctx, tc, t, C_, N_, H_):
    """Emit the per-core kernel body. t: dict of dram APs."""
    import concourse.mybir as mybir
    from concourse.bass import ts, ds

    nc = tc.nc
    dt = mybir.dt
    CT = C_ // 128          # c/d tiles (contraction over features)
    NT = N_ // 128          # token tiles (n or m)
    JT = C_ // 128          # output-feature tiles for q/k
    VW = H_ * 64            # v width (64 cols per head)
    VWC = VW + H_           # v width + content-bias columns folded in
    NCH = _chunks(N_, 512)  # n chunks for moving operand
    VCH = _chunks(VWC, 512)
    CCH = _chunks(C_, 384)  # proj output chunks (<=512, 2 banks-friendly)

    singles = ctx.enter_context(tc.tile_pool(name="singles", bufs=1))
    kh_pool = ctx.enter_context(tc.tile_pool(name="khp", bufs=2))
    exp_pool = ctx.enter_context(tc.tile_pool(name="expp", bufs=2))
    r8_pool = ctx.enter_context(tc.tile_pool(name="r8p", bufs=2))
    small = ctx.enter_context(tc.tile_pool(name="small", bufs=4))
    pf_pool = ctx.enter_context(tc.tile_pool(name="pfp", bufs=3))
    ystage = ctx.enter_context(tc.tile_pool(name="ystage", bufs=3))
    psum = ctx.enter_context(tc.tile_pool(name="psum", bufs=3, space="PSUM"))
    psum_o = ctx.enter_context(tc.tile_pool(name="psum_o", bufs=3, space="PSUM"))
    psum_y = ctx.enter_context(tc.tile_pool(name="psum_y", bufs=2, space="PSUM"))

    bf = dt.bfloat16
    f32 = dt.float32
    f32r = dt.float32r
    f8 = dt.float8e4
    exp_scale = SCALE / (MIX_UPSCALE * QK_UPSCALE * QK_UPSCALE)

    # ---- persistent SBUF tensors ----
    GP = H_ // 2            # head pairs (proj contraction tiles of 128)
    wmixT_s = singles.tile([128, CT, H_], f32, tag="wmixT")
    wproj_s = singles.tile([128, GP, C_], bf, tag="wproj")
    bprojB_s = singles.tile([128, C_], f32, tag="bprojB")

    qT_s = singles.tile([128, JT, N_], f8, tag="qT")
    kT_s = singles.tile([128, JT, N_], bf, tag="kT")
    v8_s = singles.tile([128, NT, VW], f8, tag="v8")
    cb_s = singles.tile([128, NT, H_], f32, tag="cb")
    ao_s = singles.tile([128, GP, N_], bf, tag="ao")
    c_s = singles.tile([64, H_], f32, tag="c")
    ones8_s = singles.tile([128, 2, 64], f8, tag="ones8")

    # ---- stage B: projections (inputs scoped to a pool freed afterwards) ----
    with tc.tile_pool(name="stageb", bufs=1) as sbp:
        xT_s = sbp.tile([128, CT, N_], bf, tag="xT")
        xT8_s = sbp.tile([128, CT, N_], f8, tag="xT8")
        wqT_s = sbp.tile([128, CT, C_], f8, tag="wqT")
        wkT_s = sbp.tile([128, CT, C_], f8, tag="wkT")
        wvT_s = sbp.tile([128, CT, VWC], bf, tag="wvT")
        bvB_s = sbp.tile([128, VWC], f32, tag="bvB")
        cvbias_s = sbp.tile([1, VWC], f32, tag="cvbias")
        colsumxf_s = sbp.tile([128, CT], f32, tag="colsumxf")
        colsumx_s = sbp.tile([128, CT], bf, tag="colsumx")
        cstage_s = sbp.tile([1, VWC], f32, tag="cstage")

        # per-c-tile DMAs, compute-first order, so matmul accumulation can
        # begin as soon as the first tiles land
        xT_d = t["xT"].rearrange("(t p) n -> p t n", p=128)
        wq_d = t["wqT"].rearrange("(t p) n -> p t n", p=128)
        wk_d = t["wkT"].rearrange("(t p) n -> p t n", p=128)
        wv_d = t["wvT_aug"].rearrange("(t p) n -> p t n", p=128)
        # sync carries ONLY x8 and scalar ONLY wk ahead of the first matmul;
        # everything not needed immediately issues behind them
        xT8_d = t["xT8"].rearrange("(t p) n -> p t n", p=128)
        for ct in range(CT):
            nc.scalar.dma_start(out=wkT_s[:, ct, :], in_=wk_d[:, ct, :])
            nc.sync.dma_start(out=xT8_s[:, ct, :], in_=xT8_d[:, ct, :])
        for ct in range(CT):
            nc.scalar.dma_start(out=wqT_s[:, ct, :], in_=wq_d[:, ct, :])
        for ct in range(CT):
            nc.sync.dma_start(out=xT_s[:, ct, :], in_=xT_d[:, ct, :])
        nc.scalar.dma_start(
            out=wmixT_s, in_=t["wmixT"].rearrange("(t p) n -> p t n", p=128)
        )
        for ct in range(CT):
            nc.sync.dma_start(out=wvT_s[:, ct, :], in_=wv_d[:, ct, :])
        nc.sync.dma_start(out=bvB_s, in_=t["bvB_aug"])
        nc.sync.dma_start(out=cvbias_s, in_=t["cvbias"])
        nc.sync.dma_start(out=wproj_s, in_=t["wproj64"])
        nc.sync.dma_start(out=bprojB_s, in_=t["bprojB"])
        nc.vector.memset(ones8_s, 1.0)

        # kT then qT (kT needed first for head-0 mix-scale)
        for dst, w_s in ((kT_s, wkT_s), (qT_s, wqT_s)):
            for jt in range(JT):
                for (no, nsz) in NCH:
                    ps = psum.tile([128, 512], f32, tag="ps")
                    for ct in range(0, CT, 2):
                        nc.tensor.matmul(
                            ps[:, :nsz],
                            lhsT=w_s[:, ct : ct + 2, ts(jt, 128)],
                            rhs=xT8_s[:, ct : ct + 2, ds(no, nsz)],
                            start=(ct == 0),
                            stop=(ct == CT - 2),
                            perf_mode=mybir.MatmulPerfMode.DoubleRow,
                        )
                    nc.any.tensor_copy(out=dst[:, jt, ds(no, nsz)], in_=ps[:, :nsz])

        # v (n-major, augmented with 32-cols, x32-upscaled, cast e4m3) +
        # bias add; the last H_ columns of the augmented weight are Wcb
        # (SCALE prefolded, unscaled), so the content bias falls out of the
        # same matmuls for free (ln64 is added on extraction so the exp
        # activation emits E = 64*exp(z))
        for mt in range(NT):
            for (vo, vsz) in VCH:
                ps = psum.tile([128, 512], f32, tag="ps")
                for ct in range(CT):
                    nc.tensor.matmul(
                        ps[:, :vsz],
                        lhsT=xT_s[:, ct, ts(mt, 128)],
                        rhs=wvT_s[:, ct, ds(vo, vsz)],
                        start=(ct == 0),
                        stop=(ct == CT - 1),
                    )
                if vo + vsz <= VW:
                    nc.vector.tensor_add(
                        out=v8_s[:, mt, ds(vo, vsz)],
                        in0=ps[:, :vsz],
                        in1=bvB_s[:, ds(vo, vsz)],
                    )
                else:
                    vs_v = VW - vo
                    nc.vector.tensor_add(
                        out=v8_s[:, mt, ds(vo, vs_v)],
                        in0=ps[:, :vs_v],
                        in1=bvB_s[:, ds(vo, vs_v)],
                    )
                    nc.vector.tensor_scalar_add(
                        out=cb_s[:, mt, :],
                        in0=ps[:, vs_v : vs_v + H_],
                        scalar1=LN_R,
                    )

        # colsum_v: DVE-reduce x over tokens, one-row matmul through the
        # same augmented Wv, unscale, add 1024*bv (and 1024 for the
        # denominator columns), round-trip through DRAM to transpose the
        # row onto 65 partitions as [65, H]
        for ct in range(CT):
            nc.vector.reduce_sum(
                out=colsumxf_s[:, ct : ct + 1],
                in_=xT_s[:, ct, :],
                axis=mybir.AxisListType.X,
            )
        nc.vector.tensor_copy(out=colsumx_s, in_=colsumxf_s)
        for (vo, vsz) in VCH:
            ps_cf = psum.tile([128, 512], f32, tag="ps")
            ps_c = ps_cf[0:1, :]
            for ct in range(CT):
                nc.tensor.matmul(
                    ps_c[:, :vsz],
                    lhsT=colsumx_s[:, ct : ct + 1],
                    rhs=wvT_s[:, ct, ds(vo, vsz)],
                    start=(ct == 0),
                    stop=(ct == CT - 1),
                )
            nc.vector.tensor_scalar(
                out=cstage_s[:, ds(vo, vsz)],
                in0=ps_c[:, :vsz],
                scalar1=1.0 / V_UPSCALE,
                scalar2=None,
                op0=mybir.AluOpType.mult,
            )
        nc.vector.tensor_add(
            out=cstage_s[:, :], in0=cstage_s[:, :], in1=cvbias_s[:, :]
        )
        nc.sync.dma_start(out=t["c_dram"], in_=cstage_s[0:1, 0:VW])
        nc.sync.dma_start(
            out=c_s, in_=t["c_dram"].rearrange("(h p) -> p h", p=64)
        )

    # ---- head loop (software-pipelined emission: scores(h) then AV(h-1)) ----
    def emit_scores(h, kh_t, E_t, r8_t):
        for dt_i in range(CT):
            nc.vector.tensor_scalar_mul(
                kh_t[:, dt_i, :], kT_s[:, dt_i, :], wmixT_s[:, dt_i, h : h + 1]
            )
        for mt in range(NT):
            for (no, nsz) in NCH:
                ps = psum.tile([128, 512], f32, tag="ps")
                for di in range(0, CT, 2):
                    nc.tensor.matmul(
                        ps[:, :nsz],
                        lhsT=kh_t[:, di : di + 2, ts(mt, 128)],
                        rhs=qT_s[:, di : di + 2, ds(no, nsz)],
                        start=(di == 0),
                        stop=(di == CT - 2),
                        perf_mode=mybir.MatmulPerfMode.DoubleRow,
                    )
                nc.scalar.activation(
                    out=E_t[:, mt, ds(no, nsz)],
                    in_=ps[:, :nsz],
                    func=mybir.ActivationFunctionType.Exp,
                    bias=cb_s[:, mt, h : h + 1],
                    scale=exp_scale,
                )
                nc.gpsimd.tensor_scalar_add(
                    out=r8_t[:, mt, ds(no, nsz)],
                    in0=E_t[:, mt, ds(no, nsz)],
                    scalar1=-R_UPSCALE,
                )

    def emit_av(h, r8_t, chunks=None):
        for (no, nsz) in (chunks if chunks is not None else NCH):
            # softmax denominator: all-ones stationary (dual-fp8 LDWEIGHTS
            # requires 64/128 active cols) -> S*64 replicated on 64
            # partitions; no DRAM broadcast round-trip needed
            ps_s = psum_o.tile([64, 512], f32, tag="po")
            for mt in range(0, NT, 2):
                nc.tensor.matmul(
                    ps_s[:, :nsz],
                    lhsT=ones8_s,
                    rhs=r8_t[:, mt : mt + 2, ds(no, nsz)],
                    start=(mt == 0),
                    stop=(mt == NT - 2),
                    perf_mode=mybir.MatmulPerfMode.DoubleRow,
                )
            po = psum_o.tile([64, 512], f32, tag="po")
            for mt in range(0, NT, 2):
                nc.tensor.matmul(
                    po[:, :nsz],
                    lhsT=v8_s[:, mt : mt + 2, ds(h * 64, 64)],
                    rhs=r8_t[:, mt : mt + 2, ds(no, nsz)],
                    start=(mt == 0),
                    stop=(mt == NT - 2),
                    perf_mode=mybir.MatmulPerfMode.DoubleRow,
                )
            # S = 1024 + colsum_r; reciprocal directly, all partitions
            recipB = small.tile([64, 512], f32, tag="recipB")
            nc.vector.tensor_scalar(
                out=recipB[:, :nsz],
                in0=ps_s[:, :nsz],
                scalar1=1.0 / R_UPSCALE,
                scalar2=float(N_),
                op0=mybir.AluOpType.mult,
                op1=mybir.AluOpType.add,
            )
            nc.vector.reciprocal_approx_fast(
                out=recipB[:, :nsz], in_=recipB[:, :nsz]
            )
            # fused drain: mean term + unscale
            pf = pf_pool.tile([64, 512], f32, tag="pf")
            nc.vector.tensor_scalar(
                out=pf[:, :nsz],
                in0=po[:, :nsz],
                scalar1=AV_DIV,
                scalar2=c_s[:, h : h + 1],
                op0=mybir.AluOpType.mult,
                op1=mybir.AluOpType.add,
            )
            if h % 2 == 0:
                nc.vector.tensor_mul(
                    out=ao_s[0:64, h // 2, ds(no, nsz)],
                    in0=pf[:, :nsz],
                    in1=recipB[:, :nsz],
                )
            else:
                # odd heads land on partitions 64-127 of the pair tile; DVE
                # can't shift partitions, so normalize into a temp and DMA.
                ao_tmp = small.tile([64, 512], bf, tag="ao_tmp")
                nc.vector.tensor_mul(
                    out=ao_tmp[:, :nsz], in0=pf[:, :nsz], in1=recipB[:, :nsz]
                )
                nc.sync.dma_start(
                    out=ao_s[64:128, h // 2, ds(no, nsz)], in_=ao_tmp[:, :nsz]
                )

    prev = None
    head_order = list(range(H_))
    if H_ >= 2:
        head_order[-2], head_order[-1] = head_order[-1], head_order[-2]
    for h in head_order:
        kh_t = kh_pool.tile([128, CT, N_], f8, tag="kh")
        E_t = exp_pool.tile([128, NT, N_], bf, tag="exp")
        r8_t = r8_pool.tile([128, NT, N_], f8, tag="r8")
        emit_scores(h, kh_t, E_t, r8_t)
        if prev is not None:
            emit_av(prev[0], prev[1])
        prev = (h, r8_t)

    # last head's AV immediately
    emit_av(prev[0], prev[1])

    # ---- output projection + bproj ----
    # Pairs 0..GP-2 (heads 0..H-3) are final once av(H-3)'s normalize lands;
    # their proj matmuls run right after the last AV so the PE stays dense
    # (full pstate) while the last pair's normalize + DMA-shift settle; the
    # last pair then accumulates on top from SBUF.
    if GP > 1:
        yacc_s = singles.tile([128, NT, C_], f32, tag="yacc")
        for nt in range(NT):
            for (co, csz) in CCH:
                ps = psum_y.tile([128, 512], f32, tag="psy")
                for g in range(GP - 1):
                    nc.tensor.matmul(
                        ps[:, :csz],
                        lhsT=ao_s[:, g, ts(nt, 128)],
                        rhs=wproj_s[:, g, ds(co, csz)],
                        start=(g == 0),
                        stop=(g == GP - 2),
                    )
                nc.vector.tensor_add(
                    out=yacc_s[:, nt, ds(co, csz)],
                    in0=ps[:, :csz],
                    in1=bprojB_s[:, ds(co, csz)],
                )

    y_engines = (nc.scalar, nc.gpsimd, nc.sync)
    for nt in range(NT):
        yst = ystage.tile([128, C_], bf, tag="yst")
        for (co, csz) in CCH:
            # alternate pools: scores' pool is free by now, doubling the
            # banks in flight so the DVE adds never stall the matmuls
            if (nt * len(CCH) + (co // 384)) % 2 == 0:
                ps = psum_y.tile([128, 512], f32, tag="psy")
            else:
                ps = psum.tile([128, 512], f32, tag="ps")
            nc.tensor.matmul(
                ps[:, :csz],
                lhsT=ao_s[:, GP - 1, ts(nt, 128)],
                rhs=wproj_s[:, GP - 1, ds(co, csz)],
                start=True,
                stop=True,
            )
            if GP > 1:
                nc.vector.tensor_add(
                    out=yst[:, ds(co, csz)],
                    in0=ps[:, :csz],
                    in1=yacc_s[:, nt, ds(co, csz)],
                )
            else:
                nc.vector.tensor_add(
                    out=yst[:, ds(co, csz)],
                    in0=ps[:, :csz],
                    in1=bprojB_s[:, ds(co, csz)],
                )
        y_engines[nt % 3].dma_start(out=t["y"][ts(nt, 128), :], in_=yst)


def build(C_=C, N_=N, H_=H, ncores=NCORES):
    import concourse.bacc as bacc
    import concourse.mybir as mybir
    import concourse.tile as tile

    dt = mybir.dt
    nc = bacc.Bacc(
        "TRN2", target_bir_lowering=False, debug=False, num_devices=ncores
    )
    VW = H_ * 64
    VWC = VW + H_
    t = {}
    t["xT"] = nc.dram_tensor("xT", [C_, N_], dt.bfloat16, kind="ExternalInput").ap()
    t["xT8"] = nc.dram_tensor("xT8", [C_, N_], dt.float8e4, kind="ExternalInput").ap()
    t["wqT"] = nc.dram_tensor("wqT", [C_, C_], dt.float8e4, kind="ExternalInput").ap()
    t["wkT"] = nc.dram_tensor("wkT", [C_, C_], dt.float8e4, kind="ExternalInput").ap()
    t["wvT_aug"] = nc.dram_tensor(
        "wvT_aug", [C_, VWC], dt.bfloat16, kind="ExternalInput"
    ).ap()
    t["wmixT"] = nc.dram_tensor(
        "wmixT", [C_, H_], dt.float32, kind="ExternalInput"
    ).ap()
    t["wproj64"] = nc.dram_tensor(
        "wproj64", [128, H_ // 2, C_], dt.bfloat16, kind="ExternalInput"
    ).ap()
    t["bvB_aug"] = nc.dram_tensor(
        "bvB_aug", [128, VWC], dt.float32, kind="ExternalInput"
    ).ap()
    t["cvbias"] = nc.dram_tensor(
        "cvbias", [1, VWC], dt.float32, kind="ExternalInput"
    ).ap()
    t["bprojB"] = nc.dram_tensor(
        "bprojB", [128, C_], dt.float32, kind="ExternalInput"
    ).ap()

    t["y"] = nc.dram_tensor("y", [N_, C_], dt.bfloat16, kind="ExternalOutput").ap()
    t["c_dram"] = nc.dram_tensor(
        "c_dram", [VW], dt.float32, kind="Internal"
    ).ap()

    from contextlib import ExitStack

    with tile.TileContext(nc) as tc:
        with ExitStack() as ctx:
            emit(ctx, tc, t, C_, N_, H_)
    nc.compile()
    return nc


def prep_inputs(x, Wq, Wk, Wv, bv, Wmix, Wcb, Wproj, bproj, C_=C, N_=N, H_=H):
    """Host-side: build per-core input maps from full inputs."""
    VW = H_ * 64
    VWC = VW + H_
    import ml_dtypes as _md
    F8 = _md.float8_e4m3
    wqT = np.ascontiguousarray(np.asarray(Wq, np.float32).T * QK_UPSCALE).astype(F8)
    wkT = np.ascontiguousarray(np.asarray(Wk, np.float32).T * QK_UPSCALE).astype(F8)
    wvT = np.ascontiguousarray(np.asarray(Wv, np.float32).T)  # [c, j]
    wvT_aug = np.zeros((C_, VWC), np.float32)
    bvB_aug = np.zeros((128, VWC), np.float32)
    cvbias = np.zeros((1, VWC), np.float32)
    bv = np.asarray(bv, np.float32)
    wvT_aug[:, :VW] = wvT * V_UPSCALE
    bvB_aug[:, :VW] = (bv * V_UPSCALE)[None, :]
    cvbias[0, :VW] = bv * N_
    wvT_aug[:, VW:VWC] = np.asarray(Wcb, np.float32).T * SCALE
    wmixT = np.ascontiguousarray(np.asarray(Wmix, np.float32).T) * MIX_UPSCALE
    wprojT = np.asarray(Wproj, np.float32).T  # [j, c]
    wproj64 = np.ascontiguousarray(
        wprojT.reshape(H_ // 2, 128, C_).transpose(1, 0, 2)
    ).astype(BF16)
    bprojB = np.broadcast_to(np.asarray(bproj, np.float32), (128, C_)).copy()

    shared = {
        "wqT": wqT,
        "wkT": wkT,
        "wvT_aug": wvT_aug.astype(BF16),
        "wmixT": wmixT,
        "wproj64": wproj64,
        "bvB_aug": bvB_aug,
        "cvbias": cvbias,
        "bprojB": bprojB,
    }
    x = np.asarray(x, np.float32)
    in_maps = []
    for b in range(x.shape[0]):
        m = dict(shared)
        xb = np.ascontiguousarray(x[b].T)
        m["xT"] = xb.astype(BF16)
        m["xT8"] = xb.astype(F8)
        in_maps.append(m)
    return in_maps


def kernel(x, Wq, Wk, Wv, bv, Wmix, Wcb, Wproj, bproj):
    from concourse.bass_utils import run_bass_kernel_spmd

    if "nc" not in _CACHE:
        _CACHE["nc"] = build()
    nc = _CACHE["nc"]
    in_maps = prep_inputs(x, Wq, Wk, Wv, bv, Wmix, Wcb, Wproj, bproj)
    res = run_bass_kernel_spmd(nc, in_maps, core_ids=list(range(NCORES)))
    out = np.stack([res.results[b]["y"] for b in range(len(in_maps))], axis=0)
    return out.astype(np.float32)
